# revision 3
# baseline (speedup 1.0000x reference)
"""Causal multi-head attention on 8 Trainium2 NeuronCores.

Sharding: core c handles batch b = c//2 and head-half hg = c%2 (8 of 16
heads, as 4 pairs). Per core: QKV projection (bf16 matmuls, f32 PSUM),
flash-style causal attention in transposed layout (scores_T[t, s], softmax
denominator via a ones-column appended to V), pairwise AllGather of the
(unnormalized-then-normalized) attention outputs, and a column-parallel
output projection (w_o columns sharded host-side per core parity). Host
reassembles y from the per-core [m_half, s] transposed outputs.
"""
import sys

sys.path.insert(0, "/opt/trn_rl_repo")

import numpy as np
import ml_dtypes

import concourse.bass as bass
import concourse.mybir as mybir
import concourse.tile as tile
from concourse import bacc
from concourse.bass_utils import run_bass_kernel_spmd

BF16 = ml_dtypes.bfloat16
DT = mybir.dt.bfloat16
F32 = mybir.dt.float32
EXP = mybir.ActivationFunctionType.Exp

B, S, DM, H, DK = 4, 2048, 1024, 16, 64
N_CORES = 8
N_PAIRS = 4          # head pairs per core (8 heads)
N_MCH = DM // 128    # m-chunks of the model dim (contraction for QKV proj)
REPLICA_GROUPS = [[0, 1], [2, 3], [4, 5], [6, 7]]


def build_nc(seq=S, n_pairs=N_PAIRS):
    """Build the SPMD kernel graph. seq must be a multiple of 512."""
    nst = seq // 512          # 512-wide s-tiles
    ntt_all = seq // 128      # 128-wide t-tiles
    nc = bacc.Bacc("TRN2", target_bir_lowering=False, debug=False,
                   num_devices=N_CORES)

    xT = nc.dram_tensor("xT", [DM, seq], DT, kind="ExternalInput")
    wq = nc.dram_tensor("wq", [DM, 128 * n_pairs], DT, kind="ExternalInput")
    wk = nc.dram_tensor("wk", [DM, 128 * n_pairs], DT, kind="ExternalInput")
    wv = nc.dram_tensor("wv", [DM, 128 * n_pairs], DT, kind="ExternalInput")
    # w_o^T slice: rows d (= 2 * 128*n_pairs gathered), my 512 output cols
    wo = nc.dram_tensor("wo", [2 * 128 * n_pairs, 512], DT, kind="ExternalInput")
    masks = nc.dram_tensor("masks", [4, 128, 512], DT, kind="ExternalInput")
    yT = nc.dram_tensor("yT", [512, seq], F32, kind="ExternalOutput")

    n_dch = 2 * n_pairs   # d-chunks of 128 in the gathered attention
    hw = 128 * n_pairs    # head-dim columns per core (2*n_pairs heads × 64)

    with tile.TileContext(nc) as tc:
        with (
            tc.tile_pool(name="dram", bufs=1, space="DRAM") as dram,
            tc.tile_pool(name="persist", bufs=1) as persist,
        ):
            ag_in = dram.tile([128, n_pairs, seq], DT)
            ag_out = dram.tile([2, 128, n_pairs, seq], DT)

            q_sb = persist.tile([128, n_pairs, seq], DT, tag="q")
            k_sb = persist.tile([128, n_pairs, seq], DT, tag="k")
            v_sb = persist.tile([128, ntt_all, 2 * n_pairs, 65], DT, tag="v")
            a_sb = persist.tile([128, n_pairs, seq], DT, tag="a")
            mask_sb = persist.tile([128, 4, 512], DT, tag="mask")
            wo_sb = persist.tile([128, n_dch, 512], DT, tag="wo")

            for kk in range(4):
                nc.sync.dma_start(out=mask_sb[:, kk, :], in_=masks[kk])
            nc.sync.dma_start(
                out=wo_sb[:],
                in_=wo[:].rearrange("(c p) n -> p c n", p=128),
            )
            # ones column for the softmax-denominator rows of V
            nc.vector.memset(v_sb[:, :, :, 64], 1.0)

            # ---------------- phase 1: QKV projections ----------------
            with (
                tc.tile_pool(name="ph1", bufs=1) as ph1,
                tc.tile_pool(name="psum_p", bufs=4, space="PSUM") as pp,
            ):
                xt = []
                for st in range(nst):
                    t = ph1.tile([128, N_MCH, 512], DT, tag=f"xt{st}")
                    nc.sync.dma_start(
                        out=t[:],
                        in_=xT[:].rearrange("(c p) s -> p c s", p=128)[
                            :, :, st * 512:(st + 1) * 512
                        ],
                    )
                    xt.append(t)
                wq_sb = ph1.tile([128, N_MCH, 128 * n_pairs], DT, tag="wq")
                wk_sb = ph1.tile([128, N_MCH, 128 * n_pairs], DT, tag="wk")
                wv_sb = ph1.tile([128, N_MCH, 128 * n_pairs], DT, tag="wv")
                for w_sb, w_dram in ((wq_sb, wq), (wk_sb, wk), (wv_sb, wv)):
                    nc.sync.dma_start(
                        out=w_sb[:],
                        in_=w_dram[:].rearrange("(c p) n -> p c n", p=128),
                    )

                # V projection: out[t, h*64+k] tiles, all heads at once
                for tt in range(ntt_all):
                    st, r = tt // 4, tt % 4
                    ps = pp.tile([128, hw], F32, tag="proj")
                    for c in range(N_MCH):
                        nc.tensor.matmul(
                            ps[:],
                            lhsT=xt[st][:, c, r * 128:(r + 1) * 128],
                            rhs=wv_sb[:, c, 0:hw],
                            start=(c == 0),
                            stop=(c == N_MCH - 1),
                        )
                    nc.any.tensor_copy(
                        v_sb[:, tt, :, 0:64],
                        ps[:].rearrange("p (h k) -> p h k", k=64),
                    )

                # Q^T / K^T projections: out[pair-k-rows, s]
                for pair in range(n_pairs):
                    for st in range(nst):
                        for w_sb, dst in ((wq_sb, q_sb), (wk_sb, k_sb)):
                            ps = pp.tile([128, 512], F32, tag="proj")
                            for c in range(N_MCH):
                                nc.tensor.matmul(
                                    ps[:],
                                    lhsT=w_sb[:, c, pair * 128:(pair + 1) * 128],
                                    rhs=xt[st][:, c, :],
                                    start=(c == 0),
                                    stop=(c == N_MCH - 1),
                                )
                            nc.any.tensor_copy(
                                dst[:, pair, st * 512:(st + 1) * 512], ps[:]
                            )

            # ---------------- phase 2: causal attention ----------------
            with (
                tc.tile_pool(name="psum_s", bufs=4, space="PSUM") as ps_s,
                tc.tile_pool(name="psum_av", bufs=4, space="PSUM") as ps_av,
                tc.tile_pool(name="pt", bufs=6) as p_pool,
                tc.tile_pool(name="nrm", bufs=4) as nrm,
            ):
                for pair in range(n_pairs):
                    for st in range(nst):
                        av0 = ps_av.tile([65, 512], F32, tag="av")
                        av1 = ps_av.tile([65, 512], F32, tag="av")
                        av = [av0, av1]
                        ntt = 4 * st + 4
                        pts = {}
                        # software-pipelined emission: scores(tt+1) before PV(tt)
                        for tt in range(ntt + 1):
                            if tt < ntt:
                                for h in range(2):
                                    lo = h * 64
                                    ps = ps_s.tile([128, 512], F32, tag="sc")
                                    nc.tensor.matmul(
                                        ps[:],
                                        lhsT=k_sb[lo:lo + 64, pair,
                                                  tt * 128:(tt + 1) * 128],
                                        rhs=q_sb[lo:lo + 64, pair,
                                                 st * 512:(st + 1) * 512],
                                        start=True,
                                        stop=True,
                                    )
                                    pt = p_pool.tile([128, 512], DT, tag="pt")
                                    nc.scalar.activation(pt[:], ps[:], EXP,
                                                         scale=0.125)
                                    if tt >= 4 * st:
                                        nc.vector.tensor_mul(
                                            pt[:], pt[:],
                                            mask_sb[:, tt - 4 * st, :],
                                        )
                                    pts[(tt, h)] = pt
                            if tt > 0:
                                for h in range(2):
                                    nc.tensor.matmul(
                                        av[h][:],
                                        lhsT=v_sb[:, tt - 1, 2 * pair + h, :],
                                        rhs=pts.pop((tt - 1, h))[:],
                                        start=(tt - 1 == 0),
                                        stop=(tt - 1 == ntt - 1),
                                    )
                        for h in range(2):
                            r = nrm.tile([1, 512], F32, tag="r")
                            nc.vector.reciprocal(r[:], av[h][64:65, :])
                            bb = nrm.tile([64, 512], F32, tag="b")
                            nc.gpsimd.partition_broadcast(bb[:], r[:])
                            nc.vector.tensor_mul(
                                a_sb[h * 64:(h + 1) * 64, pair,
                                     st * 512:(st + 1) * 512],
                                av[h][0:64, :],
                                bb[:],
                            )

            # ---------------- phase 3: pairwise exchange ----------------
            nc.sync.dma_start(out=ag_in[:], in_=a_sb[:])
            nc.gpsimd.collective_compute(
                "AllGather",
                mybir.AluOpType.bypass,
                replica_groups=REPLICA_GROUPS,
                ins=[ag_in.opt()],
                outs=[ag_out.opt()],
            )
            af_sb = persist.tile([128, n_dch, seq], DT, tag="af")
            for g in range(2):
                nc.sync.dma_start(
                    out=af_sb[:, g * n_pairs:(g + 1) * n_pairs, :],
                    in_=ag_out[g],
                )

            # ---------------- phase 4: output projection ----------------
            with (
                tc.tile_pool(name="psum_o", bufs=4, space="PSUM") as po,
                tc.tile_pool(name="yc", bufs=4) as ycp,
            ):
                yT_v = yT[:].rearrange("(t p) s -> p t s", p=128)
                for mt in range(4):
                    for st in range(nst):
                        ps = po.tile([128, 512], F32, tag="o")
                        for c in range(n_dch):
                            nc.tensor.matmul(
                                ps[:],
                                lhsT=wo_sb[:, c, mt * 128:(mt + 1) * 128],
                                rhs=af_sb[:, c, st * 512:(st + 1) * 512],
                                start=(c == 0),
                                stop=(c == n_dch - 1),
                            )
                        yc = ycp.tile([128, 512], F32, tag="yc")
                        nc.any.tensor_copy(yc[:], ps[:])
                        nc.sync.dma_start(
                            out=yT_v[:, mt, st * 512:(st + 1) * 512],
                            in_=yc[:],
                        )
    nc.compile()
    return nc


def _make_masks():
    p = np.arange(128)[:, None]
    f = np.arange(512)[None, :]
    return np.stack(
        [(p <= f - 128 * kk).astype(BF16) for kk in range(4)]
    )


_NC_CACHE = {}


def _get_nc(seq=S, n_pairs=N_PAIRS):
    key = (seq, n_pairs)
    if key not in _NC_CACHE:
        _NC_CACHE[key] = build_nc(seq, n_pairs)
    return _NC_CACHE[key]


def kernel(x, w_qkv, w_o):
    x = np.asarray(x, dtype=np.float32)
    w_qkv = np.asarray(w_qkv, dtype=np.float32)
    w_o = np.asarray(w_o, dtype=np.float32)

    nc = _get_nc()
    masks = _make_masks()
    in_maps = []
    for c in range(N_CORES):
        b, hg = c // 2, c % 2
        heads = slice(hg * 8, hg * 8 + 8)
        xT = np.ascontiguousarray(x[b].T).astype(BF16)
        wqc = np.ascontiguousarray(
            w_qkv[0, heads].reshape(512, DM).T).astype(BF16)
        wkc = np.ascontiguousarray(
            w_qkv[1, heads].reshape(512, DM).T).astype(BF16)
        wvc = np.ascontiguousarray(
            w_qkv[2, heads].reshape(512, DM).T).astype(BF16)
        woc = np.ascontiguousarray(
            w_o[hg * 512:(hg + 1) * 512, :].T).astype(BF16)
        in_maps.append({
            "xT": xT, "wq": wqc, "wk": wkc, "wv": wvc, "wo": woc,
            "masks": masks,
        })

    res = run_bass_kernel_spmd(nc, in_maps, list(range(N_CORES)), trace=False)

    y = np.empty((B, S, DM), dtype=np.float32)
    for c in range(N_CORES):
        b, hg = c // 2, c % 2
        y[b, :, hg * 512:(hg + 1) * 512] = res.results[c]["yT"].T
    return y


# revision 12
# speedup vs baseline: 1.5264x; 1.5264x over previous
"""Causal multi-head attention on 8 Trainium2 NeuronCores.

Sharding: core c handles batch b = c//2 and head-half hg = c%2 (8 of 16
heads, as 4 pairs). Per core: QKV projection (bf16 matmuls, f32 PSUM),
flash-style causal attention in transposed layout (scores_T[t, s], softmax
denominator via a ones-column appended to V), pairwise AllGather of the
normalized attention outputs, and a column-parallel output projection
(w_o columns sharded host-side per core parity). Host reassembles y from
the per-core [m_half, s] transposed outputs.

Loop structure: s-tiles outer; each s-tile's attention is interleaved with
the next s-tile's QKV-projection matmuls and the previous s-tile's output
projection so TensorE stays dense while ScalarE runs the exps, and the
pairwise AllGather per s-tile hides under the next s-tile's compute.
"""
import sys

sys.path.insert(0, "/opt/trn_rl_repo")

import numpy as np
import ml_dtypes

import concourse.bass as bass
import concourse.mybir as mybir
import concourse.tile as tile
from concourse import bacc
from concourse.bass_utils import run_bass_kernel_spmd

BF16 = ml_dtypes.bfloat16
DT = mybir.dt.bfloat16
F32 = mybir.dt.float32
EXP = mybir.ActivationFunctionType.Exp

B, S, DM, H, DK = 4, 2048, 1024, 16, 64
N_CORES = 8
N_PAIRS = 4          # head pairs per core (8 heads)
N_MCH = DM // 128    # m-chunks of the model dim (contraction for QKV proj)
REPLICA_GROUPS = [[0, 1], [2, 3], [4, 5], [6, 7]]


def build_nc(seq=S, n_pairs=N_PAIRS, debug_taps=False):
    """Build the SPMD kernel graph. seq must be a multiple of 512."""
    nst = seq // 512          # 512-wide s-tiles
    ntt_all = seq // 128      # 128-wide t-tiles
    nc = bacc.Bacc("TRN2", target_bir_lowering=False, debug=False,
                   num_devices=N_CORES)

    xT = nc.dram_tensor("xT", [DM, seq], DT, kind="ExternalInput")
    wq = nc.dram_tensor("wq", [DM, 128 * n_pairs], DT, kind="ExternalInput")
    wk = nc.dram_tensor("wk", [DM, 128 * n_pairs], DT, kind="ExternalInput")
    wv = nc.dram_tensor("wv", [DM, 128 * n_pairs], DT, kind="ExternalInput")
    wo = nc.dram_tensor("wo", [2 * 128 * n_pairs, 512], DT, kind="ExternalInput")
    mask128 = nc.dram_tensor("mask128", [128, 128], DT, kind="ExternalInput")
    yT = nc.dram_tensor("yT", [512, seq], F32, kind="ExternalOutput")

    n_dch = 2 * n_pairs   # d-chunks of 128 in the gathered attention
    hw = 128 * n_pairs    # head-dim columns per core (2*n_pairs heads x 64)

    with tile.TileContext(nc) as tc:
        with (
            tc.tile_pool(name="dram", bufs=1, space="DRAM") as dram,
            tc.tile_pool(name="persist", bufs=1) as persist,
            tc.tile_pool(name="psum_p", bufs=1, space="PSUM") as pp,
            tc.tile_pool(name="psum_s", bufs=2, space="PSUM") as ps_s,
            tc.tile_pool(name="psum_av", bufs=3, space="PSUM") as ps_av,
            tc.tile_pool(name="pt", bufs=4) as p_pool,
            tc.tile_pool(name="nrm", bufs=2) as nrm,
            tc.tile_pool(name="yc", bufs=3) as ycp,
            tc.tile_pool(name="stg", bufs=2) as stg,
        ):
            ag_in = dram.tile([nst, 2, 64, n_pairs, 512], DT)
            ag_out = dram.tile([nst, 2, 2, 64, n_pairs, 512], DT)

            q_sb = persist.tile([128, n_pairs, seq], DT, tag="q")
            k_sb = persist.tile([128, n_pairs, seq], DT, tag="k")
            v_sb = persist.tile([128, ntt_all, 2 * n_pairs, 65], DT, tag="v")
            af_sb = persist.tile([128, n_dch, seq], DT, tag="af")
            m_sb = persist.tile([128, 128], DT, tag="m")
            wo_sb = persist.tile([128, n_dch, 512], DT, tag="wo")
            wq_sb = persist.tile([128, N_MCH, hw], DT, tag="wq")
            wk_sb = persist.tile([128, N_MCH, hw], DT, tag="wk")
            wv_sb = persist.tile([128, N_MCH, hw], DT, tag="wv")
            xt = []
            for st in range(nst):
                t = persist.tile([128, N_MCH, 512], DT, tag=f"xt{st}")
                xt.append(t)

            nc.sync.dma_start(out=m_sb[:], in_=mask128[:])
            nc.sync.dma_start(
                out=wo_sb[:], in_=wo[:].rearrange("(c p) n -> p c n", p=128))
            for w_sb, w_dram in ((wq_sb, wq), (wk_sb, wk), (wv_sb, wv)):
                nc.sync.dma_start(
                    out=w_sb[:],
                    in_=w_dram[:].rearrange("(c p) n -> p c n", p=128))
            xT_v = xT[:].rearrange("(c p) s -> p c s", p=128)
            for st in range(nst):
                nc.sync.dma_start(
                    out=xt[st][:], in_=xT_v[:, :, st * 512:(st + 1) * 512])
            nc.vector.memset(v_sb[:, :, :, 64], 1.0)

            yT_v = yT[:].rearrange("(t p) s -> p t s", p=128)

            # ---- emission helpers (each returns a closure doing one
            # PE-dense psum-group; used to fill PE during attention) ----
            def vproj_group(tt):
                def go():
                    st, r = tt // 4, tt % 4
                    ps = pp.tile([128, hw], F32, tag="proj", name=f"psv{tt}")
                    for c in range(N_MCH):
                        nc.tensor.matmul(
                            ps[:],
                            lhsT=xt[st][:, c, r * 128:(r + 1) * 128],
                            rhs=wv_sb[:, c, 0:hw],
                            start=(c == 0), stop=(c == N_MCH - 1))
                    nc.any.tensor_copy(
                        v_sb[:, tt, :, 0:64],
                        ps[:].rearrange("p (h k) -> p h k", k=64))
                return go

            def qkproj_group(pair, st, which):
                def go():
                    w_sb, dst = ((wq_sb, q_sb), (wk_sb, k_sb))[which]
                    ps = pp.tile([128, 512], F32, tag="proj",
                                 name=f"psqk{pair}_{st}_{which}")
                    for c in range(N_MCH):
                        nc.tensor.matmul(
                            ps[:],
                            lhsT=w_sb[:, c, pair * 128:(pair + 1) * 128],
                            rhs=xt[st][:, c, :],
                            start=(c == 0), stop=(c == N_MCH - 1))
                    nc.any.tensor_copy(
                        dst[:, pair, st * 512:(st + 1) * 512], ps[:])
                return go

            def outproj_group(mt, st):
                def go():
                    ps = pp.tile([128, 512], F32, tag="proj",
                                 name=f"pso{mt}_{st}")
                    for c in range(n_dch):
                        nc.tensor.matmul(
                            ps[:],
                            lhsT=wo_sb[:, c, mt * 128:(mt + 1) * 128],
                            rhs=af_sb[:, c, st * 512:(st + 1) * 512],
                            start=(c == 0), stop=(c == n_dch - 1))
                    yc = ycp.tile([128, 512], F32, tag="yc", name=f"yc{mt}_{st}")
                    nc.any.tensor_copy(yc[:], ps[:])
                    nc.sync.dma_start(
                        out=yT_v[:, mt, st * 512:(st + 1) * 512], in_=yc[:])
                return go

            def proj_groups_for_st(st):
                gs = []
                for tt in range(4 * st, 4 * st + 4):
                    gs.append(vproj_group(tt))
                for pair in range(n_pairs):
                    for which in range(2):
                        gs.append(qkproj_group(pair, st, which))
                return gs

            if debug_taps:
                dpt = nc.dram_tensor("dpt", [4, 128, 2, 512], DT,
                                     kind="ExternalOutput")
                dav = nc.dram_tensor("dav", [2, 65, 512], F32,
                                     kind="ExternalOutput")
                dr = nc.dram_tensor("dr", [1, 2, 512], F32,
                                    kind="ExternalOutput")
                dbb = nc.dram_tensor("dbb", [64, 2, 512], F32,
                                     kind="ExternalOutput")

            # ---- attention for one (pair, st), software-pipelined ----
            def attention(pair, st, filler, stage):
                ntt = 4 * st + 4
                av0 = ps_av.tile([65, 512], F32, tag="av",
                                 name=f"av0_{pair}_{st}")
                av1 = ps_av.tile([65, 512], F32, tag="av",
                                 name=f"av1_{pair}_{st}")
                av = [av0, av1]
                pts = {}

                def scores_and_exp(tt):
                    ps = ps_s.tile([128, 2, 512], F32, tag="sc",
                                   name=f"sc{pair}_{st}_{tt}")
                    for h in range(2):
                        lo = h * 64
                        nc.tensor.matmul(
                            ps[:, h, :],
                            lhsT=k_sb[lo:lo + 64, pair,
                                      tt * 128:(tt + 1) * 128],
                            rhs=q_sb[lo:lo + 64, pair,
                                     st * 512:(st + 1) * 512],
                            start=True, stop=True)
                    pt = p_pool.tile([128, 2, 512], DT, tag="pt",
                                     name=f"pt{pair}_{st}_{tt}")
                    kk = tt - 4 * st
                    if kk < 0:
                        nc.scalar.activation(pt[:], ps[:], EXP, scale=0.125)
                    else:
                        # diagonal: zero the fully-masked cols, exp the rest,
                        # triangular mask on the boundary 128-col block
                        if kk > 0:
                            nc.vector.memset(pt[:, :, 0:kk * 128], 0.0)
                        nc.scalar.activation(
                            pt[:, :, kk * 128:512],
                            ps[:, :, kk * 128:512], EXP, scale=0.125)
                        for h in range(2):
                            nc.vector.tensor_mul(
                                pt[:, h, kk * 128:(kk + 1) * 128],
                                pt[:, h, kk * 128:(kk + 1) * 128],
                                m_sb[:])
                    if debug_taps and pair == 0 and st == 0:
                        nc.sync.dma_start(out=dpt[tt], in_=pt[:])
                    pts[tt] = pt

                def pv(tt):
                    pt = pts.pop(tt)
                    for h in range(2):
                        nc.tensor.matmul(
                            av[h][:],
                            lhsT=v_sb[:, tt, 2 * pair + h, :],
                            rhs=pt[:, h, :],
                            start=(tt == 0), stop=(tt == ntt - 1))

                for tt in range(ntt + 1):
                    if tt < ntt:
                        scores_and_exp(tt)
                    if tt > 0:
                        pv(tt - 1)
                    if filler and (tt % 2 == 1):
                        filler.pop(0)()

                if debug_taps and pair == 0 and st == 0:
                    for h in range(2):
                        avc = nrm.tile([65, 512], F32, tag="avc",
                                       name=f"avc{h}")
                        nc.vector.tensor_copy(avc[:], av[h][:])
                        nc.sync.dma_start(out=dav[h], in_=avc[:])
                # normalize: a = av[0:64] * (1/denom); denom row (psum
                # partition 64) -> sbuf -> DMA to partition 0 (the custom-DVE
                # recip and gpsimd broadcast only read partition 0 correctly)
                den = nrm.tile([65, 2, 512], F32, tag="den",
                               name=f"den{pair}_{st}")
                for h in range(2):
                    nc.vector.tensor_copy(den[64:65, h, :], av[h][64:65, :])
                den0 = nrm.tile([1, 2, 512], F32, tag="den0",
                                name=f"den0_{pair}_{st}")
                nc.sync.dma_start(out=den0[:], in_=den[64:65, :, :])
                r = nrm.tile([1, 2, 512], F32, tag="r", name=f"r{pair}_{st}")
                nc.vector.reciprocal_approx_fast(r[:], den0[:])
                bb = nrm.tile([64, 2, 512], F32, tag="b", name=f"bb{pair}_{st}")
                nc.gpsimd.partition_broadcast(bb[:], r[:])
                if debug_taps and pair == 0 and st == 0:
                    nc.sync.dma_start(out=dr[:], in_=r[:])
                    nc.sync.dma_start(out=dbb[:], in_=bb[:])
                for h in range(2):
                    nc.vector.tensor_mul(
                        stage[:, h, pair, :],
                        av[h][0:64, :], bb[:, h, :])

            # ---------------- main s-tile-outer schedule ----------------
            pending = proj_groups_for_st(0)
            while pending:
                pending.pop(0)()
            for st in range(nst):
                filler = []
                if st + 1 < nst:
                    filler += proj_groups_for_st(st + 1)
                if st >= 1:
                    for mt in range(4):
                        filler.append(outproj_group(mt, st - 1))
                stage = stg.tile([64, 2, n_pairs, 512], DT, tag="stage",
                                 name=f"stage{st}")
                for pair in range(n_pairs):
                    attention(pair, st, filler, stage)
                while filler:
                    filler.pop(0)()
                # exchange this s-tile's attention columns
                for h in range(2):
                    nc.sync.dma_start(
                        out=ag_in[st, h], in_=stage[:, h, :, :])
                nc.gpsimd.collective_compute(
                    "AllGather",
                    mybir.AluOpType.bypass,
                    replica_groups=REPLICA_GROUPS,
                    ins=[ag_in[st].opt()],
                    outs=[ag_out[st].opt()],
                )
                for g in range(2):
                    for h in range(2):
                        nc.sync.dma_start(
                            out=af_sb[h * 64:(h + 1) * 64,
                                      g * n_pairs:(g + 1) * n_pairs,
                                      st * 512:(st + 1) * 512],
                            in_=ag_out[st, g, h])
            for mt in range(4):
                outproj_group(mt, nst - 1)()

            if debug_taps:
                dq = nc.dram_tensor("dq", [128, n_pairs, seq], DT,
                                    kind="ExternalOutput")
                dk = nc.dram_tensor("dk", [128, n_pairs, seq], DT,
                                    kind="ExternalOutput")
                dv = nc.dram_tensor("dv", [128, ntt_all, 2 * n_pairs, 65], DT,
                                    kind="ExternalOutput")
                daf = nc.dram_tensor("daf", [128, n_dch, seq], DT,
                                     kind="ExternalOutput")
                for dst, src in ((dq, q_sb), (dk, k_sb), (dv, v_sb),
                                 (daf, af_sb)):
                    nc.sync.dma_start(out=dst[:], in_=src[:])
    nc.compile()
    return nc


def _make_mask128():
    p = np.arange(128)[:, None]
    f = np.arange(128)[None, :]
    return (p <= f).astype(BF16)


_NC_CACHE = {}


def _get_nc(seq=S, n_pairs=N_PAIRS):
    key = (seq, n_pairs)
    if key not in _NC_CACHE:
        _NC_CACHE[key] = build_nc(seq, n_pairs)
    return _NC_CACHE[key]


def make_in_maps(x, w_qkv, w_o):
    masks = _make_mask128()
    in_maps = []
    for c in range(N_CORES):
        b, hg = c // 2, c % 2
        heads = slice(hg * 8, hg * 8 + 8)
        in_maps.append({
            "xT": np.ascontiguousarray(x[b].T).astype(BF16),
            "wq": np.ascontiguousarray(
                w_qkv[0, heads].reshape(512, DM).T).astype(BF16),
            "wk": np.ascontiguousarray(
                w_qkv[1, heads].reshape(512, DM).T).astype(BF16),
            "wv": np.ascontiguousarray(
                w_qkv[2, heads].reshape(512, DM).T).astype(BF16),
            "wo": np.ascontiguousarray(
                w_o[hg * 512:(hg + 1) * 512, :].T).astype(BF16),
            "mask128": masks,
        })
    return in_maps


def kernel(x, w_qkv, w_o):
    x = np.asarray(x, dtype=np.float32)
    w_qkv = np.asarray(w_qkv, dtype=np.float32)
    w_o = np.asarray(w_o, dtype=np.float32)

    nc = _get_nc()
    in_maps = make_in_maps(x, w_qkv, w_o)
    res = run_bass_kernel_spmd(nc, in_maps, list(range(N_CORES)), trace=False)

    y = np.empty((B, S, DM), dtype=np.float32)
    for c in range(N_CORES):
        b, hg = c // 2, c % 2
        y[b, :, hg * 512:(hg + 1) * 512] = res.results[c]["yT"].T
    return y
